# revision 33
# baseline (speedup 1.0000x reference)
"""Trainium2 Bass kernel for nn_DistLayer (GNN message passing layer).

Computes, for full inputs (see reference):
    pa = relu(seg_mean(x[:, :128], atom_idx, 1024))[atom_idx]
    pe = relu(seg_mean(x[:, 128:], ele_idx, 100))[ele_idx]
    h  = concat([dist_feat, pa, pe], 1) @ W1 (+ b1)
    out = relu(batchnorm_train(h; gamma, beta) + x)

Note b1 provably cancels in (h - mean(h)), so it is ignored.

Strategy (8 cores, data-parallel over rows):
  - Rows sharded 25000/core; each shard bucketed by atom_idx>>7 into 8
    fixed-size 3456-row windows (pad rows are inert), so segment sums and
    the gather-back both use narrow [128,128] one-hot matmuls.
  - AllReduce #1 combines per-core segment sums [128, 1152].
  - Pooled means -> relu -> matmul with W1 halves gives per-segment
    contribution tables kept in SBUF; rows are expanded back with
    transposed one-hot matmuls accumulated straight into the h PSUM.
  - h kept resident in SBUF (bf16); BN stats via ones-matmul column sums,
    AllReduce #2, then fused affine+residual+relu output pass.
"""
import sys

sys.path.insert(0, "/opt/trn_rl_repo")

import numpy as np

import concourse.bass as bass
import concourse.mybir as mybir
import concourse.tile as tile
from concourse import bacc
from concourse.bass_utils import run_bass_kernel_spmd, axon_active

# problem constants
N = 200000
NAE = 128
NDE = 128
G = 1024
E = 100
NCORES = 8
RPC = N // NCORES          # 25000 rows per core
NW = 8                     # windows (atom segment buckets of 128)
CPW = 27                   # chunks (of 128 rows) per window
BUCKET = CPW * 128         # 3456 padded rows per window
TROWS = NW * BUCKET        # 27648 padded rows per core
T = TROWS // 128           # 216 chunks
SUMW = G + 128             # 1152: [atom sums | ele sums(padded to 128)]
EPS = 1e-5
INV_N = 1.0 / N

F32 = mybir.dt.float32
BF16 = mybir.dt.bfloat16

_CACHED_PROGRAM = None


class Cfg:
    """Size configuration; defaults = the real problem."""

    def __init__(self, rpc=RPC, cpw=CPW, gg=None, debug=None):
        self.rpc = rpc
        self.cpw = cpw
        self.bucket = cpw * 128
        self.trows = NW * self.bucket
        self.t = self.trows // 128
        assert self.t % 8 == 0
        self.inv_n = 1.0 / (rpc * NCORES)
        self.debug = debug


def _build_program(cfg=None):
    cfg = cfg or Cfg()
    CPW, TROWS, T = cfg.cpw, cfg.trows, cfg.t
    INV_N = cfg.inv_n
    dbg = (not axon_active()) if cfg.debug is None else cfg.debug
    nc = bacc.Bacc(
        "TRN2",
        target_bir_lowering=False,
        debug=dbg,
        num_devices=NCORES,
    )

    # per-core external I/O (all activations pre-converted to bf16 on host)
    xsb = nc.dram_tensor("xsb", [TROWS, 2 * NAE], BF16, kind="ExternalInput")
    dsTb = nc.dram_tensor("dsTb", [NDE, TROWS], BF16, kind="ExternalInput")
    ohra = nc.dram_tensor("ohra", [TROWS, 128], BF16, kind="ExternalInput")
    ohre = nc.dram_tensor("ohre", [TROWS, 128], BF16, kind="ExternalInput")
    ohta = nc.dram_tensor("ohta", [128, TROWS], BF16, kind="ExternalInput")
    ohte = nc.dram_tensor("ohte", [128, TROWS], BF16, kind="ExternalInput")
    w1 = nc.dram_tensor("w1", [3 * 128, 2 * NAE], F32, kind="ExternalInput")
    gb = nc.dram_tensor("gb", [1, 512], F32, kind="ExternalInput")
    rcb = nc.dram_tensor("rcb", [128, SUMW], F32, kind="ExternalInput")
    ones1 = nc.dram_tensor("ones1", [1, 128], F32, kind="ExternalInput")
    out_d = nc.dram_tensor("out", [TROWS, 2 * NAE], F32, kind="ExternalOutput")

    # internal DRAM (collective bounce buffers)
    cc1_in = nc.dram_tensor("cc1_in", [128, SUMW], F32)
    cc1_out = nc.dram_tensor("cc1_out", [128, SUMW], F32, addr_space="Shared")
    cc2_in = nc.dram_tensor("cc2_in", [1, 1024], F32)
    cc2_out = nc.dram_tensor("cc2_out", [1, 1024], F32, addr_space="Shared")

    RELU = mybir.ActivationFunctionType.Relu
    SQUARE = mybir.ActivationFunctionType.Square
    SQRT = mybir.ActivationFunctionType.Sqrt
    ISEQ = mybir.AluOpType.is_equal

    NQ = T // 4                      # quads (4-chunk groups)
    FG = 9 if NQ % 9 == 0 else NQ    # sums flush-group size in quads

    with tile.TileContext(nc) as tc:
        with (
            tc.tile_pool(name="const", bufs=1) as cp,
            tc.tile_pool(name="hcache", bufs=1) as hp,
            tc.tile_pool(name="xload", bufs=3) as xp,
            tc.tile_pool(name="dload", bufs=2) as dp,
            tc.tile_pool(name="work", bufs=2) as wp,
            tc.tile_pool(name="outp", bufs=2) as op_,
        ):
            # ---- constants into SBUF
            w1bf = []
            for i in range(3):
                tf = wp.tile([128, 256], F32, tag="w1f")
                nc.sync.dma_start(tf[:], w1[i * 128 : (i + 1) * 128, :])
                tb = cp.tile([128, 256], BF16, tag=f"w1b{i}")
                nc.scalar.copy(tb[:], tf[:])
                w1bf.append(tb)
            w1d, w1a, w1e = w1bf

            rcb_sb = cp.tile([128, SUMW], F32, tag="rcb")
            nc.sync.dma_start(rcb_sb[:], rcb[:])
            ones1_sb = cp.tile([1, 128], F32, tag="ones1")
            nc.sync.dma_start(ones1_sb[:], ones1[:])
            gb_sb = cp.tile([1, 512], F32, tag="gb")
            nc.sync.dma_start(gb_sb[:], gb[:])
            onescol = cp.tile([128, 1], BF16, tag="onescol")
            nc.vector.memset(onescol[:], 1.0)

            # ---- Stage A: local segment sums (transposed: [ae_dim, seg])
            acc = cp.tile([128, SUMW], F32, tag="acc")

            psA = tc.alloc_tile_pool(name="psA", bufs=2, space="PSUM")
            for w in range(NW):
                ps_a = psA.tile([128, 128], F32, tag="ps_a")
                ps_e = psA.tile([128, 128], F32, tag="ps_e")
                done = 0
                while done < CPW:
                    b = min(8, CPW - done)
                    t0 = w * CPW + done
                    rows = slice(t0 * 128, (t0 + b) * 128)
                    xq = xp.tile([128, 8, 256], BF16, tag="xq")
                    nc.sync.dma_start(
                        xq[:, 0:b, :],
                        xsb[rows, :].rearrange("(n p) m -> p n m", p=128),
                    )
                    ra = wp.tile([128, 8, 128], BF16, tag="ra")
                    nc.scalar.dma_start(
                        ra[:, 0:b, :],
                        ohra[rows, :].rearrange("(n p) m -> p n m", p=128),
                    )
                    re = wp.tile([128, 8, 128], BF16, tag="re")
                    nc.gpsimd.dma_start(
                        re[:, 0:b, :],
                        ohre[rows, :].rearrange("(n p) m -> p n m", p=128),
                    )
                    for j in range(b):
                        nc.tensor.matmul(
                            ps_a[:], lhsT=xq[:, j, 0:128], rhs=ra[:, j, :],
                            start=(done + j == 0), stop=(done + j == CPW - 1),
                        )
                        nc.tensor.matmul(
                            ps_e[:], lhsT=xq[:, j, 128:256], rhs=re[:, j, :],
                            start=(done + j == 0), stop=(done + j == CPW - 1),
                        )
                    done += b
                nc.vector.tensor_copy(acc[:, w * 128 : (w + 1) * 128], ps_a[:])
                if w == 0:
                    nc.vector.tensor_copy(acc[:, G : G + 128], ps_e[:])
                else:
                    nc.vector.tensor_add(
                        acc[:, G : G + 128], acc[:, G : G + 128], ps_e[:]
                    )
            psA.release()
            psH = tc.alloc_tile_pool(name="psH", bufs=2, space="PSUM")
            psS = tc.alloc_tile_pool(name="psS", bufs=1, space="PSUM")

            # ---- AllReduce #1 (segment sums)
            nc.sync.dma_start(cc1_in[:], acc[:])
            nc.gpsimd.collective_compute(
                "AllReduce",
                mybir.AluOpType.add,
                replica_groups=[list(range(NCORES))],
                ins=[cc1_in[:]],
                outs=[cc1_out[:]],
            )
            nc.sync.dma_start(acc[:], cc1_out[:])

            # ---- tables: relu(mean) @ W1 part, kept in SBUF (bf16)
            nc.vector.tensor_mul(acc[:], acc[:], rcb_sb[:])
            rmeans = cp.tile([128, SUMW], BF16, tag="rmeans")
            nc.scalar.activation(rmeans[:], acc[:], RELU)

            tbl_a = cp.tile([128, NW, 256], BF16, tag="tbl_a")
            for blk in range(NW):
                pst = psH.tile([128, 512], F32, tag="psbc")
                nc.tensor.matmul(
                    pst[:, 0:256],
                    lhsT=rmeans[:, blk * 128 : (blk + 1) * 128],
                    rhs=w1a[:],
                    start=True,
                    stop=True,
                )
                nc.scalar.copy(tbl_a[:, blk, :], pst[:, 0:256])
            tbl_e = cp.tile([128, 256], BF16, tag="tbl_e")
            pst = psH.tile([128, 512], F32, tag="psbc")
            nc.tensor.matmul(
                pst[:, 0:256], lhsT=rmeans[:, G : G + 128], rhs=w1e[:],
                start=True, stop=True,
            )
            nc.scalar.copy(tbl_e[:], pst[:, 0:256])

            # ---- Stage C: h = dsT.T@W1d + onehotT_a.T@tbl_a + onehotT_e.T@tbl_e
            hbuf = hp.tile([128, T, 256], BF16, tag="H")
            acc_s1 = cp.tile([1, 512], F32, tag="acc_s1")
            acc_s2 = cp.tile([1, 512], F32, tag="acc_s2")

            ps1 = ps2 = None
            dq = oa = oe = None
            for q in range(NQ):
                if q % 2 == 0:
                    cols = slice(q * 512, (q + 2) * 512)
                    dq = dp.tile([128, 1024], BF16, tag="dq")
                    nc.sync.dma_start(dq[:, 0 : min(1024, TROWS - q * 512)],
                                      dsTb[:, cols])
                    oa = dp.tile([128, 1024], BF16, tag="oa")
                    nc.gpsimd.dma_start(oa[:, 0 : min(1024, TROWS - q * 512)],
                                      ohta[:, cols])
                    oe = dp.tile([128, 1024], BF16, tag="oe")
                    nc.gpsimd.dma_start(oe[:, 0 : min(1024, TROWS - q * 512)],
                                      ohte[:, cols])
                off = (q % 2) * 512
                ps4 = psH.tile([128, 4, 256], F32, tag="ps4")
                for k in range(4):
                    t = q * 4 + k
                    w = t // CPW
                    sl = slice(off + k * 128, off + (k + 1) * 128)
                    nc.tensor.matmul(
                        ps4[:, k, :], lhsT=dq[:, sl], rhs=w1d[:],
                        start=True, stop=False,
                    )
                    nc.tensor.matmul(
                        ps4[:, k, :], lhsT=oa[:, sl], rhs=tbl_a[:, w, :],
                        start=False, stop=False,
                    )
                    nc.tensor.matmul(
                        ps4[:, k, :], lhsT=oe[:, sl], rhs=tbl_e[:],
                        start=False, stop=True,
                    )
                hs = hbuf[:, q * 4 : (q + 1) * 4, :]
                nc.scalar.copy(hs, ps4[:])


                gfirst = q % FG == 0
                glast = q % FG == FG - 1 or q == NQ - 1
                if gfirst:
                    ps1 = psS.tile([1, 512], F32, tag="ps1")
                    ps2 = psS.tile([1, 512], F32, tag="ps2")
                for hf in range(2):
                    sl2 = hbuf[:, q * 4 + 2 * hf : q * 4 + 2 * hf + 2, :]
                    hq = wp.tile([128, 2, 256], BF16, tag="hq")
                    nc.vector.tensor_mul(hq[:], sl2, sl2)
                    nc.tensor.matmul(
                        ps1[:], lhsT=onescol[:],
                        rhs=sl2.rearrange("p n m -> p (n m)"),
                        start=(gfirst and hf == 0), stop=(glast and hf == 1),
                    )
                    nc.tensor.matmul(
                        ps2[:], lhsT=onescol[:],
                        rhs=hq[:].rearrange("p n m -> p (n m)"),
                        start=(gfirst and hf == 0), stop=(glast and hf == 1),
                    )
                if glast:
                    if q < FG:
                        nc.vector.tensor_copy(acc_s1[:], ps1[:])
                        nc.vector.tensor_copy(acc_s2[:], ps2[:])
                    else:
                        nc.vector.tensor_add(acc_s1[:], acc_s1[:], ps1[:])
                        nc.vector.tensor_add(acc_s2[:], acc_s2[:], ps2[:])

            # ---- AllReduce #2 (batchnorm sums) + affine constants
            sdt = cp.tile([1, 1024], F32, tag="sdt")
            nc.vector.tensor_copy(sdt[:, 0:512], acc_s1[:])
            nc.vector.tensor_copy(sdt[:, 512:1024], acc_s2[:])
            nc.sync.dma_start(cc2_in[:], sdt[:])
            nc.gpsimd.collective_compute(
                "AllReduce",
                mybir.AluOpType.add,
                replica_groups=[list(range(NCORES))],
                ins=[cc2_in[:]],
                outs=[cc2_out[:]],
            )
            nc.sync.dma_start(sdt[:], cc2_out[:])

            s1f = cp.tile([1, 256], F32, tag="s1f")
            nc.vector.tensor_add(s1f[:], sdt[:, 0:256], sdt[:, 256:512])
            s2f = cp.tile([1, 256], F32, tag="s2f")
            nc.vector.tensor_add(s2f[:], sdt[:, 512:768], sdt[:, 768:1024])
            mu = cp.tile([1, 256], F32, tag="mu")
            nc.scalar.mul(mu[:], s1f[:], INV_N)
            ex2 = cp.tile([1, 256], F32, tag="ex2")
            nc.scalar.mul(ex2[:], s2f[:], INV_N)
            mu2 = cp.tile([1, 256], F32, tag="mu2")
            nc.vector.tensor_mul(mu2[:], mu[:], mu[:])
            var = cp.tile([1, 256], F32, tag="var")
            nc.vector.tensor_sub(var[:], ex2[:], mu2[:])
            veps = cp.tile([1, 1], F32, tag="veps")
            nc.vector.memset(veps[:], EPS)
            std = cp.tile([1, 256], F32, tag="std")
            nc.scalar.activation(std[:], var[:], SQRT, bias=veps[:])
            rstd = cp.tile([1, 256], F32, tag="rstd")
            nc.vector.reciprocal(rstd[:], std[:])
            ab = cp.tile([1, 512], F32, tag="ab")
            nc.vector.tensor_mul(ab[:, 0:256], rstd[:], gb_sb[:, 0:256])
            mua = cp.tile([1, 256], F32, tag="mua")
            nc.vector.tensor_mul(mua[:], mu[:], ab[:, 0:256])
            nc.vector.tensor_sub(ab[:, 256:512], gb_sb[:, 256:512], mua[:])

            psb = psH.tile([128, 512], F32, tag="psbc")
            nc.tensor.matmul(
                psb[:], lhsT=ones1_sb[:], rhs=ab[:], start=True, stop=True
            )
            A_b8 = cp.tile([128, 8, 256], BF16, tag="A_b8")
            B_b8 = cp.tile([128, 8, 256], BF16, tag="B_b8")
            for j in range(8):
                nc.scalar.copy(A_b8[:, j, :], psb[:, 0:256])
                nc.scalar.copy(B_b8[:, j, :], psb[:, 256:512])

            # ---- Stage E: out = relu(h * A + B + x)
            NO = T // 8
            for o in range(NO):
                rows = slice(o * 1024, (o + 1) * 1024)
                xb8 = xp.tile([128, 8, 256], BF16, tag="xq")
                nc.sync.dma_start(
                    xb8[:], xsb[rows, :].rearrange("(n p) m -> p n m", p=128)
                )
                m8 = wp.tile([128, 8, 256], BF16, tag="m8")
                nc.vector.tensor_mul(
                    m8[:], hbuf[:, o * 8 : (o + 1) * 8, :], A_b8[:]
                )
                nc.vector.tensor_add(m8[:], m8[:], xb8[:])
                nc.vector.tensor_add(m8[:], m8[:], B_b8[:])
                for h2 in range(2):
                    ot = op_.tile([128, 4, 256], F32, tag="ot")
                    msl = m8[:, h2 * 4 : (h2 + 1) * 4, :]
                    nc.scalar.activation(ot[:], msl, RELU)
                    r2 = slice(o * 1024 + h2 * 512, o * 1024 + (h2 + 1) * 512)
                    nc.sync.dma_start(
                        out_d[r2, :].rearrange("(n p) m -> p n m", p=128),
                        ot[:],
                    )

            psS.release()
            psH.release()

    nc.compile()
    return nc


def _get_program():
    global _CACHED_PROGRAM
    if _CACHED_PROGRAM is None:
        _CACHED_PROGRAM = _build_program()
    return _CACHED_PROGRAM


def _plan_core(x_s, d_s, a_s, e_s, cfg=None):
    """Bucket one core's rows by atom window; return padded arrays + row map."""
    import ml_dtypes

    cfg = cfg or Cfg()
    TROWS, BUCKET, RPC, T = cfg.trows, cfg.bucket, cfg.rpc, cfg.t
    bucket = (a_s >> 7).astype(np.int64)
    order = np.argsort(bucket, kind="stable")
    counts = np.bincount(bucket, minlength=NW)
    if counts.max() > BUCKET:
        raise RuntimeError(f"window overflow: {counts.max()} > {BUCKET}")

    BF = ml_dtypes.bfloat16
    xp_ = np.zeros((TROWS, 2 * NAE), BF)
    dp_ = np.zeros((TROWS, NDE), np.float32)
    awp = np.full(TROWS, -1, np.int64)
    ewp = np.full(TROWS, -1, np.int64)
    pos = np.empty(RPC, np.int64)

    start = 0
    for w in range(NW):
        k = counts[w]
        rows = order[start : start + k]
        start += k
        b = w * BUCKET
        xp_[b : b + k] = x_s[rows].astype(BF)
        dp_[b : b + k] = d_s[rows]
        awp[b : b + k] = a_s[rows] - 128 * w
        ewp[b : b + k] = e_s[rows]
        pos[rows] = np.arange(b, b + k)

    dsTb = np.ascontiguousarray(dp_.T).astype(BF)
    ar = np.arange(128, dtype=np.int64)
    ohra = (awp[:, None] == ar[None, :]).astype(BF)
    ohre = (ewp[:, None] == ar[None, :]).astype(BF)
    ohta = np.ascontiguousarray(ohra.T)
    ohte = np.ascontiguousarray(ohre.T)
    return xp_, dsTb, ohra, ohre, ohta, ohte, pos


def _prepare(x, dist_feat, atom_idx, ele_idx, W1, gamma, beta, cfg=None):
    """Shard+plan all cores; returns (in_maps, positions)."""
    cfg = cfg or Cfg()
    x = np.ascontiguousarray(np.asarray(x, dtype=np.float32))
    dist_feat = np.ascontiguousarray(np.asarray(dist_feat, dtype=np.float32))
    atom_idx = np.asarray(atom_idx).astype(np.int64)
    ele_idx = np.asarray(ele_idx).astype(np.int64)
    W1 = np.ascontiguousarray(np.asarray(W1, dtype=np.float32))
    gamma = np.asarray(gamma, dtype=np.float32)
    beta = np.asarray(beta, dtype=np.float32)

    rc = np.zeros((SUMW,), np.float32)
    rc[:G] = 1.0 / np.maximum(np.bincount(atom_idx, minlength=G), 1.0)
    rc[G : G + E] = 1.0 / np.maximum(np.bincount(ele_idx, minlength=E), 1.0)
    rcb = np.ascontiguousarray(np.broadcast_to(rc, (128, SUMW))).astype(np.float32)
    ones1 = np.ones((1, 128), np.float32)
    gbv = np.concatenate([gamma, beta]).reshape(1, 512).astype(np.float32)

    in_maps = []
    positions = []
    for c in range(NCORES):
        sl = slice(c * cfg.rpc, (c + 1) * cfg.rpc)
        xsb, dsTb, ohra, ohre, ohta, ohte, pos = _plan_core(
            x[sl], dist_feat[sl], atom_idx[sl], ele_idx[sl], cfg
        )
        positions.append(pos)
        in_maps.append(
            {
                "xsb": xsb,
                "dsTb": dsTb,
                "ohra": ohra,
                "ohre": ohre,
                "ohta": ohta,
                "ohte": ohte,
                "w1": W1,
                "gb": gbv,
                "rcb": rcb,
                "ones1": ones1,
            }
        )
    return in_maps, positions


def kernel(x, dist_feat, atom_idx, ele_idx, W1, b1, gamma, beta, num_graphs, num_eles):
    assert int(num_graphs) == G and int(num_eles) == E
    assert np.asarray(x).shape == (N, 2 * NAE)

    nc = _get_program()
    in_maps, positions = _prepare(x, dist_feat, atom_idx, ele_idx, W1, gamma, beta)
    res = run_bass_kernel_spmd(nc, in_maps, core_ids=list(range(NCORES)))

    out = np.empty((N, 2 * NAE), np.float32)
    for c in range(NCORES):
        dev = res.results[c]["out"]
        out[c * RPC : (c + 1) * RPC] = dev[positions[c]]
    return out
